# revision 54
# baseline (speedup 1.0000x reference)
"""CARAFE content-aware upsampling kernel for Trainium2 (8 NeuronCores).

x(4,256,64,64) -> 1x1 down(64) -> 3x3 enc(100) -> softmax(25 taps) ->
reassembly + pixel-shuffle x2 -> 1x1 out(256).  Output (4,256,128,128).

Sharding: (batch n, H-half) = 8 shards; each core computes 32 low-res rows.

v2 design (per core):
  A) t = W_down@x + b_down, fp32r matmuls, t stored fp16 (65 rows incl ones).
  B) enc conv pix-major: stationary t-window (65,128pix), moving we (65,100)
     -> e psum (128pix, 100ch); Act exp -> kern (128,16,160) with channels
     padded to 32-blocks (5 wr-groups); softmax via 2-stage free-dim reduce.
  C) y = W_out@x fp32r (no bias - folded into D via softmax-sum=1), yT fp16.
  D) per row-pair T1 transposes kern -> KT ch-major (96/64 part, 72 w-pad);
     per row 5 wr-window transposes (own psum bank each) -> one 5-dim DVE
     gather copy -> KR (68, 32, 100); per 8-row batch ONE slanted DMA builds
     the banded B (68, 8,5,288) [partition-pitch+2 stride]; 10 psum-
     accumulated matmuls per row (yT stationary, B-valid-cols moving);
     out copy adds b_out (Act Identity bias / DVE tensor_tensor), fp16 out.
"""
import sys

for _p in ("/opt/trn_rl_repo",):
    if _p not in sys.path:
        sys.path.insert(0, _p)

import numpy as np

N, C, H, W = 4, 256, 64, 64
D, KUP = 2, 5
CM, E, OC = 64, 100, 256
HH = 32          # output rows per core
RS = HH + 4      # x slab rows (2-halo)
TR = HH + 2      # t rows (1-halo)
WP = W + 4       # padded width
ECH = 160        # padded enc channels (5 wr-groups x 32)
BW = 288         # B columns per dy block (2 i x 144); w+4 slots x2 +jj

_CACHE = {}
_DEBUG = False


def _build_program():
    if "nc" in _CACHE:
        return _CACHE["nc"]

    import concourse.bacc as bacc
    import concourse.mybir as mybir
    import concourse.tile as tile
    from concourse import bass
    from bass_rust import AP as RAP

    F32, F16, F32R = mybir.dt.float32, mybir.dt.float16, mybir.dt.float32r
    PSUM = bass.MemorySpace.PSUM
    AF = mybir.ActivationFunctionType

    nc = bacc.Bacc("TRN2", target_bir_lowering=False, debug=False, num_devices=8)

    xs_d = nc.dram_tensor("xs", [2, 128, RS, WP], F32R, kind="ExternalInput")
    wd_d = nc.dram_tensor("wd", [2, 128, CM], F32R, kind="ExternalInput")
    bd_d = nc.dram_tensor("bd", [1, CM], F32R, kind="ExternalInput")
    we_d = nc.dram_tensor("we", [CM + 1, 9, E], F16, kind="ExternalInput")
    wo_d = nc.dram_tensor("wo", [2, 128, OC], F32R, kind="ExternalInput")
    bo_d = nc.dram_tensor("bo", [128, 2], F32, kind="ExternalInput")
    vm_d = nc.dram_tensor("vm", [1, RS, WP], F32R, kind="ExternalInput")
    id_d = nc.dram_tensor("idt", [128, 128], F16, kind="ExternalInput")
    id2_d = nc.dram_tensor("id2", [128, 20], F16, kind="ExternalInput")
    zb_d = nc.dram_tensor("zb", [68, 8 * 5 * BW], F16, kind="ExternalInput")
    out_d = nc.dram_tensor("out", [2, 128, HH, 2, 128], F16, kind="ExternalOutput")
    if _DEBUG:
        dt_d = nc.dram_tensor("dbg_t", [CM + 1, TR, WP], F16, kind="ExternalOutput")
        dk_d = nc.dram_tensor("dbg_kern", [128, 16, ECH], F16, kind="ExternalOutput")
        dy_d = nc.dram_tensor("dbg_yT", [WP, RS, OC], F16, kind="ExternalOutput")
        dr_d = nc.dram_tensor("dbg_kr", [68, HH, E], F16, kind="ExternalOutput")
        da_d = nc.dram_tensor("dbg_kta", [96, HH, 72], F16, kind="ExternalOutput")
        db_d = nc.dram_tensor("dbg_bt", [68, HH // 4, 5, BW], F16, kind="ExternalOutput")

    with tile.TileContext(nc) as tc:
        with (
            tc.tile_pool(name="const", bufs=1) as cp,
            tc.tile_pool(name="sm", bufs=4) as smp,
            tc.tile_pool(name="rop", bufs=3) as rop,
        ):
            xsb = cp.tile([128, 2, RS, WP], F32R, tag="xsb")
            xs0 = xsb[:, 0]
            xs1 = xsb[:, 1]
            wd0 = cp.tile([128, CM], F32R, tag="wd0")
            wd1 = cp.tile([128, CM], F32R, tag="wd1")
            bd_t = cp.tile([1, CM], F32R, tag="bd")
            we_t = cp.tile([CM + 1, 9, E], F16, tag="we")
            wo0 = cp.tile([128, OC], F32R, tag="wo0")
            wo1 = cp.tile([128, OC], F32R, tag="wo1")
            bo_t = cp.tile([128, 2], F32, tag="bo")
            vm_t = cp.tile([1, RS, WP], F32R, tag="vm")
            id_t = cp.tile([128, 128], F16, tag="idt")
            id2_t = cp.tile([128, 20], F16, tag="id2")
            t_t = cp.tile([CM + 1, TR, WP], F16, tag="t")
            kern = cp.tile([128, 16, ECH], F16, tag="kern")
            yT = cp.tile([WP, RS, OC], F16, tag="yT")
            kta = cp.tile([96, HH, 72], F16, tag="kta")
            ktb = cp.tile([64, HH, 72], F16, tag="ktb")
            kr = cp.tile([68, HH, E], F16, tag="kr")
            bt0 = cp.tile([68, 8, 5, BW], F16, tag="bt0")
            bt1 = cp.tile([68, 8, 5, BW], F16, tag="bt1")

            # small consts first so phase A can start as soon as xs0 lands
            def xs_chunk_dma(r0, r1):
                nr = r1 - r0
                src = RAP(xs_d[:].tensor, r0 * WP,
                          [[RS * WP, 128], [128 * RS * WP, 2], [1, nr * WP]])
                dst = RAP(xsb[:].tensor, r0 * WP,
                          [[2 * RS * WP, 128], [RS * WP, 2], [1, nr * WP]])
                nc.sync.dma_start(dst, src)

            nc.sync.dma_start(wd0[:], wd_d[0])
            xs_chunk_dma(0, 5)
            nc.sync.dma_start(wd1[:], wd_d[1])
            nc.sync.dma_start(bd_t[:], bd_d[:])
            nc.sync.dma_start(vm_t[:], vm_d[:])
            nc.sync.dma_start(we_t[:], we_d[:])
            # rest of xs in row-chunks so phase A pipelines with the loads
            for r0, r1 in ((5, 11), (11, 18), (18, 25), (25, RS)):
                xs_chunk_dma(r0, r1)
            nc.sync.dma_start(wo0[:], wo_d[0])
            nc.sync.dma_start(wo1[:], wo_d[1])
            nc.sync.dma_start(bo_t[:], bo_d[:])
            nc.sync.dma_start(id_t[:], id_d[:])
            nc.sync.dma_start(id2_t[:], id2_d[:])
            # off-band zeros for the banded B come from a DRAM zeros blob
            # (band positions are identical every batch, written once)
            nc.sync.dma_start(bt0[:].rearrange("p a b c -> p (a b c)"), zb_d[:])
            nc.sync.dma_start(bt1[:].rearrange("p a b c -> p (a b c)"), zb_d[:])
            nc.vector.memset(t_t[CM : CM + 1, :, :], 1.0)
            nc.gpsimd.memset(kta[:], 0.0)
            nc.gpsimd.memset(ktb[:], 0.0)
            nc.gpsimd.memset(kern[:], 0.0)

            # ---- phases A+B interleaved: A chunks (t = W_down@x + b_down)
            # feed B chunks (enc conv pix-major + exp + softmax) as their
            # t rows land.  A shares the epp psum buffers (same shape/tags).
            # Stationary APs allow 1 free dim -> one matmul per output row;
            # each row-half gets its own full psum bank (2KB) to avoid
            # same-bank multi-group writes.
            a_chunks = [(0, 4), (4, 6), (10, 6), (16, 6), (22, 6), (28, 6)]

            def emit_a_chunk(epp, j):
                r0, nr = a_chunks[j]
                tp = epp.tile([64, 512], F32, tag="ep0" if j % 2 == 0 else "ep1")
                tpv = tp[:, 0 : nr * WP].rearrange("p (a b) -> p a b", a=nr)
                nc.tensor.matmul(tpv, wd0[:], xs0[:, 1 + r0 : 1 + r0 + nr, :],
                                 start=True, stop=False)
                nc.tensor.matmul(tpv, wd1[:], xs1[:, 1 + r0 : 1 + r0 + nr, :],
                                 start=False, stop=False)
                nc.tensor.matmul(tpv, bd_t[:], vm_t[:, 1 + r0 : 1 + r0 + nr, :],
                                 start=False, stop=True)
                nc.vector.tensor_copy(t_t[0:CM, r0 : r0 + nr, :], tpv)

            with (
                tc.tile_pool(name="ep", bufs=3, space=PSUM) as epp,
                tc.tile_pool(name="t1", bufs=1, space=PSUM) as t1p,
            ):
                emit_a_chunk(epp, 0)
                emit_a_chunk(epp, 1)
                a_at = {1: 2, 3: 3, 6: 4, 9: 5}
                for s in range(16):
                    if s in a_at:
                        emit_a_chunk(epp, a_at[s])
                    ep0 = epp.tile([64, 512], F32, tag="ep0")
                    ep1 = epp.tile([64, 512], F32, tag="ep1")
                    for half, ep in ((0, ep0), (1, ep1)):
                        r = 2 * s + half
                        for tap in range(9):
                            ty, tx = tap // 3, tap % 3
                            nc.tensor.matmul(
                                ep[:, 0:E],
                                t_t[:, r + ty, 1 + tx : 1 + tx + W],
                                we_t[:, tap, :],
                                start=(tap == 0), stop=(tap == 8),
                            )
                    # exp -> kern slot, channels spread to 32-blocks
                    slot = kern[:, s, :]
                    for half, ep in ((0, ep0), (1, ep1)):
                        dst = RAP(slot.tensor, slot.offset + half * 64 * 16 * ECH,
                                  [[16 * ECH, 64], [32, 5], [1, 20]])
                        nc.scalar.activation(dst, ep[:, 0:E], AF.Exp)
                    # softmax over the 25 taps (grp, dy) per subpixel p
                    kv = RAP(slot.tensor, slot.offset,
                             [[16 * ECH, 128], [32, 5], [1, 4], [4, 5]])  # (p, grp, p4, dy)
                    # one XY-reduce over (grp, dy) per subpixel p
                    kvr = RAP(slot.tensor, slot.offset,
                              [[16 * ECH, 128], [1, 4], [32, 5], [4, 5]])
                    ssum = smp.tile([128, 4, 1, 1], F32, tag="ssum")
                    nc.vector.tensor_reduce(ssum[:], kvr, mybir.AxisListType.XY,
                                            mybir.AluOpType.add)
                    rinv = smp.tile([128, 4], F32, tag="rinv")
                    nc.vector.reciprocal(rinv[:], ssum[:].squeeze().squeeze())
                    rb = RAP(rinv[:].tensor, rinv[:].offset,
                             [[4, 128], [0, 5], [1, 4], [0, 5]])
                    nc.vector.tensor_tensor(kv, kv, rb, mybir.AluOpType.mult)
                    # T1: transpose this kern slot to ch-major KT right away
                    pa = t1p.tile([96, 128], F16, tag="pa")
                    pb = t1p.tile([64, 128], F16, tag="pb")
                    nc.tensor.transpose(pa[:], kern[:, s, 0:96], id_t[:, 0:128])
                    nc.tensor.transpose(pb[:], kern[:, s, 96:160], id_t[:, 0:128])
                    nc.scalar.activation(
                        kta[:, 2 * s : 2 * s + 2, 4 : 4 + W],
                        pa[:].rearrange("p (a b) -> p a b", a=2), AF.Copy)
                    nc.vector.tensor_copy(
                        ktb[:, 2 * s : 2 * s + 2, 4 : 4 + W],
                        pb[:].rearrange("p (a b) -> p a b", a=2))

            # ---- phase D: wr-transposes + KR + slant B-DMA + reassembly ----
            NB = 4          # row batches
            BR = HH // NB   # rows per batch = 8
            grp_base = [(kta, 0), (kta, 32), (kta, 64), (ktb, 0), (ktb, 32)]

            def emit_wr_row(wpool, r):
                pw = wpool.tile([68, 5, 1024], F16, tag="pw")
                for wr in range(5):
                    src_t, b = grp_base[wr]
                    nc.tensor.transpose(
                        pw[:, wr, 0:20],
                        src_t[b : b + 20, r, wr : wr + 68],
                        id2_t[b : b + 20, 0:20])
                v_src = pw[:, :, 0:20].rearrange("p w (d i j) -> p d i w j",
                                                 d=5, i=2, j=2)
                v_dst = kr[:, r, :].rearrange("p (d i w j) -> p d i w j",
                                              d=5, i=2, w=5, j=2)
                nc.vector.tensor_copy(v_dst, v_src)

            def emit_b_dma(bts, k):
                # issued from the (otherwise idle) Pool queue so it never
                # serializes behind out-DMAs on the SP sequencer
                bt = bt0 if k % 2 == 0 else bt1
                src_ap = kr[:, k * BR, :]
                src = RAP(src_ap.tensor, src_ap.offset,
                          [[HH * E, 68], [10, 10 * BR], [1, 10]])
                dst_ap = bt[:]
                dst = RAP(dst_ap.tensor, dst_ap.offset,
                          [[BR * 5 * BW + 2, 68], [144, 10 * BR], [1, 10]])
                nc.sync.dma_start(dst, src)
                bts[k] = bt

            def emit_mm_row(psO, bts, k, rb, ro0, ro1):
                h = k * BR + rb
                bt = bts[k]
                for cf in range(2):
                    rp = psO.tile([128, 256], F32, tag="rp")
                    for dy in range(5):
                        mv_ap = bt[:, rb, dy, :]
                        mv = RAP(mv_ap.tensor, mv_ap.offset,
                                 [[BR * 5 * BW, 68], [144, 2], [1, 128]])
                        # moving offset +8: valid cols (w+4) in 4..67 -> 8..135
                        mv = RAP(mv.tensor, mv.offset + 8, mv.ap)
                        nc.tensor.matmul(
                            rp[:], yT[0:68, h + dy, 128 * cf : 128 * (cf + 1)], mv,
                            start=(dy == 0), stop=(dy == 4))
                    ro = ro0 if cf == 0 else ro1
                    if rb % 2 == 0:
                        nc.scalar.activation(ro[:, rb, :], rp[:], AF.Identity,
                                             bias=bo_t[:, cf : cf + 1])
                    else:
                        bb = bo_t[:, cf : cf + 1].to_broadcast([128, 256])
                        nc.vector.tensor_tensor(ro[:, rb, :], rp[:], bb,
                                                mybir.AluOpType.add)

            bts = {}
            with tc.tile_pool(name="wp", bufs=1, space=PSUM) as wpool:
                # ---- phase C (yT = (W_out @ x)^T fp16, no bias), with
                # batch-0 wr rows interleaved: the C matmuls keep PE busy
                # between each wr->KR-copy chain ----
                with tc.tile_pool(name="yp", bufs=3, space=PSUM) as ypp:
                    for r in range(RS):
                        if r % 3 == 0 and r // 3 < BR:
                            emit_wr_row(wpool, r // 3)
                        yp = ypp.tile([WP, OC], F32, tag="yp")
                        nc.tensor.matmul(yp[:], xs0[:, r, :], wo0[:],
                                         start=True, stop=False)
                        nc.tensor.matmul(yp[:], xs1[:, r, :], wo1[:],
                                         start=False, stop=True)
                        if r % 2 == 0:
                            nc.vector.tensor_copy(yT[:, r, :], yp[:])
                        else:
                            nc.scalar.activation(yT[:, r, :], yp[:], AF.Copy)
                        if r == 3 * BR - 2:
                            emit_b_dma(bts, 0)

                with tc.tile_pool(name="po", bufs=2, space=PSUM) as psO:
                    ro_prev = None
                    for k in range(NB):
                        ro0 = rop.tile([128, BR, 256], F16, tag="ro0")
                        ro1 = rop.tile([128, BR, 256], F16, tag="ro1")
                        for rb in range(BR):
                            if k + 1 < NB:
                                emit_wr_row(wpool, (k + 1) * BR + rb)
                            emit_mm_row(psO, bts, k, rb, ro0, ro1)
                            if rb % 2 == 1:
                                h0, h1 = k * BR + rb - 1, k * BR + rb + 1
                                nc.sync.dma_start(out_d[0, :, h0:h1, :, :],
                                                  ro0[:, rb - 1 : rb + 1, :])
                                nc.sync.dma_start(out_d[1, :, h0:h1, :, :],
                                                  ro1[:, rb - 1 : rb + 1, :])
                        if k + 1 < NB:
                            emit_b_dma(bts, k + 1)
                        ro_prev = (ro0, ro1)
                if _DEBUG:
                    nc.sync.dma_start(dt_d[:], t_t[:])
                    nc.sync.dma_start(dk_d[:], kern[:])
                    nc.sync.dma_start(dy_d[:], yT[:])
                    nc.sync.dma_start(dr_d[:], kr[:])
                    nc.sync.dma_start(da_d[:], kta[:])
                    nc.sync.dma_start(db_d[:], bt1[:])

    nc.compile()
    _CACHE["nc"] = nc
    return nc


def _host_inputs(x, W_down, b_down, W_enc, b_enc, W_out, b_out):
    wd = np.ascontiguousarray(W_down.T.reshape(2, 128, CM), np.float32)
    bd = np.ascontiguousarray(b_down[None, :], np.float32)
    # we[cm, tap(ty,tx), wr*20 + dy*4 + p] = W_enc[(dy*5 + 4-wr)*4 + p, cm, ty, tx]
    we = np.zeros((CM + 1, 9, E), np.float16)
    ch_enc = np.empty(E, np.int32)
    for wr in range(5):
        for dy in range(5):
            for p in range(4):
                ch_enc[wr * 20 + dy * 4 + p] = (dy * 5 + (4 - wr)) * 4 + p
    for tap in range(9):
        ty, tx = tap // 3, tap % 3
        we[:CM, tap, :] = W_enc[ch_enc, :, ty, tx].T.astype(np.float16)
    we[CM, 4, :] = b_enc[ch_enc].astype(np.float16)
    wo = np.ascontiguousarray(W_out.T.reshape(2, 128, OC), np.float32)
    bo = np.ascontiguousarray(b_out.reshape(2, 128).T, np.float32)  # (128, cf)
    idt = np.eye(128, dtype=np.float16)
    id2 = np.zeros((128, 20), np.float16)
    for b in (0, 32, 64, 96):
        id2[b : b + 20] = np.eye(20, dtype=np.float16)
    zb = np.zeros((68, 8 * 5 * BW), np.float16)

    in_maps = []
    for core in range(8):
        n, h0 = core // 2, (core % 2) * HH
        xs = np.zeros((C, RS, WP), np.float32)
        vm = np.zeros((1, RS, WP), np.float32)
        lo, hi = max(0, h0 - 2), min(H, h0 + HH + 2)
        xs[:, lo - (h0 - 2) : hi - (h0 - 2), 2 : 2 + W] = x[n, :, lo:hi, :]
        vm[0, lo - (h0 - 2) : hi - (h0 - 2), 2 : 2 + W] = 1.0
        in_maps.append({
            "xs": xs.reshape(2, 128, RS, WP),
            "wd": wd, "bd": bd, "we": we, "wo": wo, "bo": bo,
            "vm": vm, "idt": idt, "id2": id2, "zb": zb,
        })
    return in_maps


def kernel(x, W_down, b_down, W_enc, b_enc, W_out, b_out):
    from concourse.bass_utils import run_bass_kernel_spmd

    nc = _build_program()
    in_maps = _host_inputs(np.asarray(x, np.float32), np.asarray(W_down, np.float32),
                           np.asarray(b_down, np.float32), np.asarray(W_enc, np.float32),
                           np.asarray(b_enc, np.float32), np.asarray(W_out, np.float32),
                           np.asarray(b_out, np.float32))
    res = run_bass_kernel_spmd(nc, in_maps, list(range(8)))
    full = np.empty((N, C, 2 * H, 2 * W), np.float32)
    for core in range(8):
        n, half = core // 2, core % 2
        arr = res.results[core]["out"].astype(np.float32)
        arr = arr.reshape(2 * 128, HH * 2, 2 * W)
        full[n, :, half * 64 : (half + 1) * 64, :] = arr
    return full
